# revision 25
# baseline (speedup 1.0000x reference)
"""Multi-head attention (B=2, S=2048, D=1024, H=16) on 8 TRN2 NeuronCores.

Sharding (Megatron-style, hardcoded):
  - batch b = core // 4  (2 groups of 4 cores)
  - head group g = core % 4 -> heads [4g, 4g+4), feature slice F = 256 rows
    of w_q/w_k/w_v (column-parallel) and 256 columns of w_out (row-parallel).
Each core computes a full [S, D] partial of the output for its batch; the
host sums the 4 partials per batch and adds b_out.

v2 design (measured PE cost ~= N * (K/128) * (ceil(M/64)*64/128) cycles):
  - scores row-tiled: per head-pair, two K=64 matmuls at partition bases
    0/64 run at 2x column rate (110ns vs 246ns per [*,512]).
  - ctx col-tiled: per pair, two M=64 matmuls (heads at psum partition
    halves) at 2x rate; softmax denominators are separate M=1 col-tiled
    ones-matmuls accumulated in a dedicated psum bank (rows 0/64).
  - V^T computed directly in the projection (x stationary, w moving), so
    no PE transposes; V^T tiles interleave per-kt with pair-0 attention.
  - exp split across engines: ACT exact exp for most key-tiles, DVE
    "Schraudolph" exp (f32 -> int16 affine+convert, bitcast bf16) for
    DVE_KTS; softmax normalization makes the ~1.8% rms sawtooth error
    mostly wash out (adds ~1% output rel err on a 2e-2 budget).
  - softmax denominator broadcast via gpsimd partition_broadcast, then
    DVE reciprocal + multiply; normalization applied to ctx in SBUF.
"""

import os

import numpy as np

import concourse.bass as bass
import concourse.tile as tile
from concourse import bacc, mybir
from concourse.bass_utils import run_bass_kernel_spmd

B, S, D, H, DK = 2, 2048, 1024, 16, 64
N_CORES = 8
GROUPS = 4              # head-groups (cores per batch)
HL = H // GROUPS        # heads per core = 4
F = HL * DK             # feature slice per core = 256
FT = F // 128           # f-tiles (head pairs) per core = 2
DT = D // 128           # d-tiles (contraction) = 8
KT = S // 128           # 128-wide key tiles = 16
NQ = S // 512           # 512-wide query quarters = 4
TT = S // 128           # 128-wide t-tiles for out-proj = 16

F32 = mybir.dt.float32
BF16 = mybir.dt.bfloat16
I16 = mybir.dt.int16
AFT = mybir.ActivationFunctionType
ALU = mybir.AluOpType

# which key-tiles go to the DVE Schraudolph exp (rest on ACT exact exp)
DVE_KTS = frozenset(
    int(x) for x in os.environ.get("DVE_KTS", "4,9,14").split(",") if x != ""
)
# bf16 Schraudolph: i16 = round(x*a + b); bitcast i16 -> bf16 ~= exp(x)
EXP_A = 128.0 / float(np.log(2.0))
EXP_B = float(os.environ.get("EXP_B", 127.0 * 128.0 - 7.37))

_CACHE = {}
LAST_RESULTS = None  # BassKernelResults of the most recent run (for test.py)


def _build():
    nc = bacc.Bacc("TRN2", target_bir_lowering=False, debug=False,
                   num_devices=N_CORES)

    xq = nc.declare_dram_parameter("xq_t", [DT, 128, S], BF16, isOutput=False)
    xk = nc.declare_dram_parameter("xk_t", [DT, 128, S], BF16, isOutput=False)
    xv = nc.declare_dram_parameter("xv_t", [DT, 128, S], BF16, isOutput=False)
    wq = nc.declare_dram_parameter("wq_t", [128, DT, F], BF16, isOutput=False)
    wk = nc.declare_dram_parameter("wk_t", [128, DT, F], BF16, isOutput=False)
    wv = nc.declare_dram_parameter("wv_t", [128, DT, F], BF16, isOutput=False)
    bq = nc.declare_dram_parameter("bq", [128, FT], F32, isOutput=False)
    bk = nc.declare_dram_parameter("bk", [128, FT], F32, isOutput=False)
    bv = nc.declare_dram_parameter("bv_row", [1, F], F32, isOutput=False)
    wo = nc.declare_dram_parameter("wo_t", [128, FT, D], BF16, isOutput=False)
    out = nc.declare_dram_parameter("out_p", [S, D], BF16, isOutput=True)
    KDBG = bool(os.environ.get("KDBG"))
    if KDBG:
        dbg_k = nc.declare_dram_parameter("dbg_k", [128, FT, S], BF16, isOutput=True)
        dbg_q = nc.declare_dram_parameter("dbg_q", [128, FT, S], BF16, isOutput=True)
        dbg_vt = nc.declare_dram_parameter("dbg_vt", [128, KT, F], BF16, isOutput=True)
        dbg_ctx = nc.declare_dram_parameter("dbg_ctx", [128, FT, S], BF16, isOutput=True)


    with tile.TileContext(nc) as tc:
        with (
            tc.tile_pool(name="const", bufs=1) as const,
            tc.tile_pool(name="acts", bufs=1) as acts,
            tc.tile_pool(name="xpool", bufs=8) as xpool,
            tc.tile_pool(name="xvpool", bufs=1) as xvpool,
            tc.tile_pool(name="wpool", bufs=1) as wpool,
            tc.tile_pool(name="ppool", bufs=10) as ppool,
            tc.tile_pool(name="norm", bufs=2) as norm,
            tc.tile_pool(name="opool", bufs=3) as opool,
        ):
            # ---- constants / weights ----
            ones_bf = const.tile([128, 64], BF16, tag="ones")
            b_sb = {}
            for name, bp in (("k", bk), ("q", bq)):
                b_sb[name] = const.tile([128, FT], F32, tag=f"b{name}",
                                        name=f"b{name}_sb")
                nc.sync.dma_start(out=b_sb[name][:], in_=bp[:])
            bv_r = const.tile([1, F], F32, tag="bvr")
            nc.sync.dma_start(out=bv_r[:], in_=bv[:])
            bv_bc = const.tile([128, F], F32, tag="bvb")
            nc.gpsimd.partition_broadcast(bv_bc[:], bv_r[:])

            w_ts = {}
            for name, wp in (("k", wk), ("q", wq), ("v", wv)):
                w_ts[name] = wpool.tile([128, DT, F], BF16, tag=f"w{name}",
                                        name=f"w{name}_sb")
            wo_sb = wpool.tile([128, FT, D], BF16, tag="wo")
            nc.sync.dma_start(out=w_ts["k"][:], in_=wk[:])
            nc.vector.memset(ones_bf[:], 1.0)

            # persistent activations (pair-major: head h at partition
            # 64*(h%2), pair slot h//2)
            k_sb = acts.tile([128, FT, S], BF16, tag="pk")
            q_sb = acts.tile([128, FT, S], BF16, tag="pq")
            vt_sb = acts.tile([128, KT, F], BF16, tag="vt")
            ctx_sb = acts.tile([128, FT, S], BF16, tag="ctx")

            # xv resident tiles (stationary operands of the V^T projection)
            xv_ts = []
            for dt in range(DT):
                t = xvpool.tile([128, S], BF16, tag=f"xv{dt}", name=f"xv{dt}")
                xv_ts.append(t)

            # ---- phase A: k proj, V^T proj, q proj ----
            def proj(name, x_d, dst, psA):
                w_t = w_ts[name]
                banks = [psA.tile([128, 512], F32, tag="pp", name=f"pp{i}")
                         for i in range(8)]
                for dt in range(DT):
                    x_t = xpool.tile([128, S], BF16, tag="x")
                    nc.sync.dma_start(out=x_t[:], in_=x_d[dt])
                    for fi in range(FT):
                        lhsT = w_t[:, dt, fi * 128:(fi + 1) * 128]
                        for tb in range(4):
                            nc.tensor.matmul(
                                banks[fi * 4 + tb][:], lhsT,
                                x_t[:, tb * 512:(tb + 1) * 512],
                                start=(dt == 0), stop=(dt == DT - 1),
                            )
                if name == "k":
                    nc.sync.dma_start(out=w_ts["q"][:], in_=wq[:])
                    nc.sync.dma_start(out=w_ts["v"][:], in_=wv[:])
                else:
                    for d2 in range(DT):
                        nc.sync.dma_start(out=xv_ts[d2][:], in_=xv[d2])
                    nc.sync.dma_start(out=wo_sb[:], in_=wo[:])
                for fi in range(FT):
                    for tb in range(4):
                        if tb % 2 == 0:
                            nc.vector.tensor_scalar_add(
                                out=dst[:, fi, tb * 512:(tb + 1) * 512],
                                in0=banks[fi * 4 + tb][:],
                                scalar1=b_sb[name][:, fi:fi + 1],
                            )
                        else:
                            nc.scalar.activation(
                                out=dst[:, fi, tb * 512:(tb + 1) * 512],
                                in_=banks[fi * 4 + tb][:],
                                func=AFT.Identity,
                                bias=b_sb[name][:, fi:fi + 1],
                            )

            warmup = const.tile([1, 8], F32, tag="wrm")
            nc.vector.memset(warmup[:], 0.0)
            nc.scalar.activation(warmup[:], warmup[:], AFT.Exp)
            with tc.tile_pool(name="psA", bufs=8, space="PSUM") as psA:
                proj("k", xk, k_sb, psA)

            with tc.tile_pool(name="psA2", bufs=8, space="PSUM") as psA2:
                proj("q", xq, q_sb, psA2)

            # V^T projection (x stationary, w moving); xv lands last
            with tc.tile_pool(name="psV", bufs=2, space="PSUM") as psVp:
                for kt in range(KT):
                    vps = psVp.tile([128, F], F32, tag="v", bufs=2, name="vps")
                    for dt in range(DT):
                        nc.tensor.matmul(
                            vps[:], xv_ts[dt][:, kt * 128:(kt + 1) * 128],
                            w_ts["v"][:, dt, :],
                            start=(dt == 0), stop=(dt == DT - 1),
                        )
                    nc.vector.tensor_tensor(
                        out=vt_sb[:, kt, :], in0=vps[:], in1=bv_bc[:],
                        op=ALU.add)

            if KDBG:
                nc.sync.dma_start(out=dbg_k[:], in_=k_sb[:])
                nc.sync.dma_start(out=dbg_q[:], in_=q_sb[:])

            # ---- phase B: quarter-major attention + interleaved out-proj ----
            psB_stack = tc.tile_pool(name="psS", bufs=2, space="PSUM")
            psS = psB_stack.__enter__()
            psC_cm = tc.tile_pool(name="psC", bufs=1, space="PSUM")
            psC = psC_cm.__enter__()
            psD_cm = tc.tile_pool(name="psD", bufs=2, space="PSUM")
            psD = psD_cm.__enter__()

            def outproj_piece(tt, j):
                ob = psO.tile([128, 512], F32, tag="ob", name="ob")
                js = slice(j * 512, (j + 1) * 512)
                for fi in range(FT):
                    nc.tensor.matmul(
                        ob[:], ctx_sb[:, fi, tt * 128:(tt + 1) * 128],
                        wo_sb[:, fi, js], start=(fi == 0), stop=(fi == FT - 1))
                o_t = opool.tile([128, 512], BF16, tag="o", name="o_t")
                nc.vector.tensor_copy(o_t[:], ob[:])
                nc.sync.dma_start(out=out[tt * 128:(tt + 1) * 128, js],
                                  in_=o_t[:])

            def attn_quarter(P, Q, pieces=()):
                h0, h1 = 2 * P, 2 * P + 1
                qs = slice(Q * 512, (Q + 1) * 512)
                d_ps = psD.tile([128, 512], F32, tag="d", name="d_ps")
                ctx_ps = psC.tile([128, 512], F32, tag="ctx", name="ctx_ps")
                pieces = list(pieces)

                def scores(kt):
                    kts = slice(kt * 128, (kt + 1) * 128)
                    s2 = psS.tile([128, 1024], F32, tag="s", name="s2")
                    nc.tensor.matmul(s2[:, 0:512], k_sb[0:64, P, kts],
                                     q_sb[0:64, P, qs], start=True, stop=True)
                    nc.tensor.matmul(s2[:, 512:1024], k_sb[64:128, P, kts],
                                     q_sb[64:128, P, qs], start=True, stop=True)
                    return s2

                def consume(kt, p2):
                    # d + ctx for kt (exp(kt) finished >= one period ago)
                    st, sp = (kt == 0), (kt == KT - 1)
                    nc.tensor.matmul(d_ps[0:64, :], ones_bf[:],
                                     p2[:, 0:512], start=st, stop=sp)
                    nc.tensor.matmul(d_ps[64:128, :], ones_bf[:],
                                     p2[:, 512:1024], start=st, stop=sp,
                                     skip_group_check=True)
                    nc.tensor.matmul(ctx_ps[0:64, :],
                                     vt_sb[:, kt, 64 * h0:64 * h0 + 64],
                                     p2[:, 0:512], start=st, stop=sp)
                    nc.tensor.matmul(ctx_ps[64:128, :],
                                     vt_sb[:, kt, 64 * h1:64 * h1 + 64],
                                     p2[:, 512:1024], start=st, stop=sp,
                                     skip_group_check=True)

                s2 = scores(0)
                prev = None  # (kt, p2) whose d/ctx are deferred one iter
                for kt in range(KT):
                    p2 = ppool.tile([128, 1024], BF16, tag="p", name="p2")
                    if kt in DVE_KTS:
                        nc.vector.tensor_scalar(
                            out=p2.bitcast(I16)[:], in0=s2[:], scalar1=EXP_A,
                            scalar2=EXP_B, op0=ALU.mult, op1=ALU.add)
                    else:
                        nc.scalar.activation(p2[:], s2[:], AFT.Exp)
                    if kt + 1 < KT:
                        s2 = scores(kt + 1)
                    if pieces and kt % 4 == 3:
                        outproj_piece(*pieces.pop(0))
                    if prev is not None:
                        consume(*prev)
                    prev = (kt, p2)
                consume(*prev)
                for piece in pieces:
                    outproj_piece(*piece)
                # normalize: M=64 denominators already span each head's 64
                # partitions -> reciprocal + aligned multiplies, no broadcast
                linv = norm.tile([128, 512], F32, tag="linv", name="linv")
                nc.vector.reciprocal_approx_fast(linv[:], d_ps[:])
                nc.vector.tensor_tensor(
                    out=ctx_sb[0:64, P, qs], in0=ctx_ps[0:64, :],
                    in1=linv[0:64, :], op=ALU.mult)
                nc.vector.tensor_tensor(
                    out=ctx_sb[64:128, P, qs], in0=ctx_ps[64:128, :],
                    in1=linv[64:128, :], op=ALU.mult)

            psO_cm = tc.tile_pool(name="psO", bufs=1, space="PSUM")
            psO = psO_cm.__enter__()
            for Q in range(NQ):
                if Q == 0:
                    prev = []
                else:
                    prev = [(tt, j) for tt in range(4 * (Q - 1), 4 * Q)
                            for j in range(2)]
                attn_quarter(0, Q, prev[:4])
                attn_quarter(1, Q, prev[4:])
            # last quarter's out-projection tail (psS banks are free now);
            # copies alternate ACT/DVE to halve the drain latency
            for tt in range(4 * (NQ - 1), 4 * NQ):
                for j in range(2):
                    obw = psS.tile([128, 1024], F32, tag="s", name="s2")
                    ob = obw[:, 0:512]
                    js = slice(j * 512, (j + 1) * 512)
                    for fi in range(FT):
                        nc.tensor.matmul(
                            ob, ctx_sb[:, fi, tt * 128:(tt + 1) * 128],
                            wo_sb[:, fi, js], start=(fi == 0),
                            stop=(fi == FT - 1))
                    o_t = opool.tile([128, 512], BF16, tag="o", name="o_t")
                    if j == 0:
                        nc.scalar.copy(out=o_t[:], in_=ob)
                    else:
                        nc.vector.tensor_copy(o_t[:], ob)
                    nc.sync.dma_start(out=out[tt * 128:(tt + 1) * 128, js],
                                      in_=o_t[:])

            if KDBG:
                nc.sync.dma_start(out=dbg_vt[:], in_=vt_sb[:])
                nc.sync.dma_start(out=dbg_ctx[:], in_=ctx_sb[:])
            psO_cm.__exit__(None, None, None)
            psD_cm.__exit__(None, None, None)
            psC_cm.__exit__(None, None, None)
            psB_stack.__exit__(None, None, None)

    nc.compile()
    return nc


def get_program():
    if "nc" not in _CACHE:
        _CACHE["nc"] = _build()
    return _CACHE["nc"]


def _bf(a):
    import ml_dtypes
    return a.astype(ml_dtypes.bfloat16)


def prep_in_maps(query_tensor, key_tensor, value_tensor, w_q, b_q, w_k, b_k,
                 w_v, b_v, w_out, b_out):
    """Per-core input dicts. Core c: batch c//4, feature rows [256*(c%4), ...)."""
    f32 = np.float32
    scale = f32(1.0 / np.sqrt(DK))

    def xt(x, b):  # [S, D] -> [DT, 128, S]
        return _bf(np.ascontiguousarray(
            np.asarray(x[b], f32).T.reshape(DT, 128, S)))

    xs = {"xq_t": [xt(query_tensor, b) for b in range(B)],
          "xk_t": [xt(key_tensor, b) for b in range(B)],
          "xv_t": [xt(value_tensor, b) for b in range(B)]}

    def wt(w, g, s=f32(1.0)):  # rows [256g, 256g+256) of w -> [128, DT, F]
        sl = np.asarray(w[256 * g:256 * (g + 1), :], f32) * s  # [F, D]
        return _bf(np.ascontiguousarray(
            sl.T.reshape(DT, 128, F).transpose(1, 0, 2)))

    def bt(b_, g, s=f32(1.0)):  # [128, FT]
        sl = np.asarray(b_[256 * g:256 * (g + 1)], f32) * s
        return np.ascontiguousarray(sl.reshape(FT, 128).T)

    def wot(w, g):  # cols [256g, 256g+256) of w_out -> [128, FT, D]
        sl = np.asarray(w[:, 256 * g:256 * (g + 1)], f32)  # [D, F]
        return _bf(np.ascontiguousarray(
            sl.T.reshape(FT, 128, D).transpose(1, 0, 2)))

    in_maps = []
    for c in range(N_CORES):
        b, g = divmod(c, GROUPS)
        bv_sl = np.asarray(b_v[256 * g:256 * (g + 1)], f32).reshape(1, F)
        in_maps.append({
            "xq_t": xs["xq_t"][b], "xk_t": xs["xk_t"][b], "xv_t": xs["xv_t"][b],
            "wq_t": wt(w_q, g, scale), "wk_t": wt(w_k, g), "wv_t": wt(w_v, g),
            "bq": bt(b_q, g, scale), "bk": bt(b_k, g),
            "bv_row": np.ascontiguousarray(bv_sl),
            "wo_t": wot(w_out, g),
        })
    return in_maps


def kernel(query_tensor, key_tensor, value_tensor, w_q, b_q, w_k, b_k,
           w_v, b_v, w_out, b_out):
    global LAST_RESULTS
    nc = get_program()
    in_maps = prep_in_maps(query_tensor, key_tensor, value_tensor, w_q, b_q,
                           w_k, b_k, w_v, b_v, w_out, b_out)
    res = run_bass_kernel_spmd(nc, in_maps, list(range(N_CORES)),
                               tmpdir=os.environ.get("BASS_TMPDIR"))
    LAST_RESULTS = res
    b_out = np.asarray(b_out, np.float32)
    out = np.empty((B, S, D), np.float32)
    for b in range(B):
        import ml_dtypes
        acc = np.asarray(res.results[4 * b]["out_p"]).view(ml_dtypes.bfloat16).astype(np.float32)
        for g in range(1, GROUPS):
            acc = acc + np.asarray(res.results[4 * b + g]["out_p"]).view(
                ml_dtypes.bfloat16).astype(np.float32)
        out[b] = acc + b_out
    return out


# revision 26
# speedup vs baseline: 1.0233x; 1.0233x over previous
"""Multi-head attention (B=2, S=2048, D=1024, H=16) on 8 TRN2 NeuronCores.

Sharding (Megatron-style, hardcoded):
  - batch b = core // 4  (2 groups of 4 cores)
  - head group g = core % 4 -> heads [4g, 4g+4), feature slice F = 256 rows
    of w_q/w_k/w_v (column-parallel) and 256 columns of w_out (row-parallel).
Each core computes a full [S, D] partial of the output for its batch; the
host sums the 4 partials per batch and adds b_out.

v2 design (measured PE cost ~= N * (K/128) * (ceil(M/64)*64/128) cycles):
  - scores row-tiled: per head-pair, two K=64 matmuls at partition bases
    0/64 run at 2x column rate (110ns vs 246ns per [*,512]).
  - ctx col-tiled: per pair, two M=64 matmuls (heads at psum partition
    halves) at 2x rate; softmax denominators are separate M=1 col-tiled
    ones-matmuls accumulated in a dedicated psum bank (rows 0/64).
  - V^T computed directly in the projection (x stationary, w moving), so
    no PE transposes; V^T tiles interleave per-kt with pair-0 attention.
  - exp split across engines: ACT exact exp for most key-tiles, DVE
    "Schraudolph" exp (f32 -> int16 affine+convert, bitcast bf16) for
    DVE_KTS; softmax normalization makes the ~1.8% rms sawtooth error
    mostly wash out (adds ~1% output rel err on a 2e-2 budget).
  - softmax denominator broadcast via gpsimd partition_broadcast, then
    DVE reciprocal + multiply; normalization applied to ctx in SBUF.
"""

import os

import numpy as np

import concourse.bass as bass
import concourse.tile as tile
from concourse import bacc, mybir
from concourse.bass_utils import run_bass_kernel_spmd

B, S, D, H, DK = 2, 2048, 1024, 16, 64
N_CORES = 8
GROUPS = 4              # head-groups (cores per batch)
HL = H // GROUPS        # heads per core = 4
F = HL * DK             # feature slice per core = 256
FT = F // 128           # f-tiles (head pairs) per core = 2
DT = D // 128           # d-tiles (contraction) = 8
KT = S // 128           # 128-wide key tiles = 16
NQ = S // 512           # 512-wide query quarters = 4
TT = S // 128           # 128-wide t-tiles for out-proj = 16

F32 = mybir.dt.float32
BF16 = mybir.dt.bfloat16
I16 = mybir.dt.int16
AFT = mybir.ActivationFunctionType
ALU = mybir.AluOpType

# which key-tiles go to the DVE Schraudolph exp (rest on ACT exact exp)
DVE_KTS = frozenset(
    int(x) for x in os.environ.get("DVE_KTS", "4,9,14").split(",") if x != ""
)
# bf16 Schraudolph: i16 = round(x*a + b); bitcast i16 -> bf16 ~= exp(x)
EXP_A = 128.0 / float(np.log(2.0))
EXP_B = float(os.environ.get("EXP_B", 127.0 * 128.0 - 7.37))

_CACHE = {}
LAST_RESULTS = None  # BassKernelResults of the most recent run (for test.py)


def _build():
    nc = bacc.Bacc("TRN2", target_bir_lowering=False, debug=False,
                   num_devices=N_CORES)

    xq = nc.declare_dram_parameter("xq_t", [DT, 128, S], BF16, isOutput=False)
    xk = nc.declare_dram_parameter("xk_t", [DT, 128, S], BF16, isOutput=False)
    xv = nc.declare_dram_parameter("xv_t", [DT, 128, S], BF16, isOutput=False)
    wq = nc.declare_dram_parameter("wq_t", [128, DT, F], BF16, isOutput=False)
    wk = nc.declare_dram_parameter("wk_t", [128, DT, F], BF16, isOutput=False)
    wv = nc.declare_dram_parameter("wv_t", [128, DT, F], BF16, isOutput=False)
    bq = nc.declare_dram_parameter("bq", [128, FT], F32, isOutput=False)
    bk = nc.declare_dram_parameter("bk", [128, FT], F32, isOutput=False)
    bv = nc.declare_dram_parameter("bv_row", [1, F], F32, isOutput=False)
    wo = nc.declare_dram_parameter("wo_t", [128, FT, D], BF16, isOutput=False)
    out = nc.declare_dram_parameter("out_p", [S, D], BF16, isOutput=True)
    KDBG = bool(os.environ.get("KDBG"))
    if KDBG:
        dbg_k = nc.declare_dram_parameter("dbg_k", [128, FT, S], BF16, isOutput=True)
        dbg_q = nc.declare_dram_parameter("dbg_q", [128, FT, S], BF16, isOutput=True)
        dbg_vt = nc.declare_dram_parameter("dbg_vt", [128, KT, F], BF16, isOutput=True)
        dbg_ctx = nc.declare_dram_parameter("dbg_ctx", [128, FT, S], BF16, isOutput=True)


    with tile.TileContext(nc) as tc:
        with (
            tc.tile_pool(name="const", bufs=1) as const,
            tc.tile_pool(name="acts", bufs=1) as acts,
            tc.tile_pool(name="xpool", bufs=8) as xpool,
            tc.tile_pool(name="xvpool", bufs=1) as xvpool,
            tc.tile_pool(name="wpool", bufs=1) as wpool,
            tc.tile_pool(name="ppool", bufs=10) as ppool,
            tc.tile_pool(name="norm", bufs=2) as norm,
            tc.tile_pool(name="opool", bufs=3) as opool,
        ):
            # ---- constants / weights (wk first: it gates the first matmul) ----
            w_ts = {}
            for name, wp in (("k", wk), ("q", wq), ("v", wv)):
                w_ts[name] = wpool.tile([128, DT, F], BF16, tag=f"w{name}",
                                        name=f"w{name}_sb")
            wo_sb = wpool.tile([128, FT, D], BF16, tag="wo")
            nc.sync.dma_start(out=w_ts["k"][:], in_=wk[:])
            ones_bf = const.tile([128, 64], BF16, tag="ones")
            nc.vector.memset(ones_bf[:], 1.0)
            b_sb = {}
            for name, bp in (("k", bk), ("q", bq)):
                b_sb[name] = const.tile([128, FT], F32, tag=f"b{name}",
                                        name=f"b{name}_sb")
                nc.sync.dma_start(out=b_sb[name][:], in_=bp[:])
            bv_r = const.tile([1, F], F32, tag="bvr")
            nc.sync.dma_start(out=bv_r[:], in_=bv[:])
            bv_bc = const.tile([128, F], F32, tag="bvb")
            nc.gpsimd.partition_broadcast(bv_bc[:], bv_r[:])

            # persistent activations (pair-major: head h at partition
            # 64*(h%2), pair slot h//2)
            k_sb = acts.tile([128, FT, S], BF16, tag="pk")
            q_sb = acts.tile([128, FT, S], BF16, tag="pq")
            vt_sb = acts.tile([128, KT, F], BF16, tag="vt")
            ctx_sb = acts.tile([128, FT, S], BF16, tag="ctx")

            # xv resident tiles (stationary operands of the V^T projection)
            xv_ts = []
            for dt in range(DT):
                t = xvpool.tile([128, S], BF16, tag=f"xv{dt}", name=f"xv{dt}")
                xv_ts.append(t)

            # ---- phase A: k proj, V^T proj, q proj ----
            prefetched = {}

            def proj(name, x_d, dst, psA):
                w_t = w_ts[name]
                banks = [psA.tile([128, 512], F32, tag="pp", name=f"pp{i}")
                         for i in range(8)]
                for dt in range(DT):
                    if name == "q" and dt == 0:
                        x_t = prefetched["xq0"]
                    else:
                        x_t = xpool.tile([128, S], BF16, tag="x")
                        nc.sync.dma_start(out=x_t[:], in_=x_d[dt])
                    if name == "k" and dt == 5:
                        t0 = xpool.tile([128, S], BF16, tag="x", name="xq0pf")
                        nc.sync.dma_start(out=t0[:], in_=xq[0])
                        prefetched["xq0"] = t0
                    for fi in range(FT):
                        lhsT = w_t[:, dt, fi * 128:(fi + 1) * 128]
                        for tb in range(4):
                            nc.tensor.matmul(
                                banks[fi * 4 + tb][:], lhsT,
                                x_t[:, tb * 512:(tb + 1) * 512],
                                start=(dt == 0), stop=(dt == DT - 1),
                            )
                if name == "k":
                    nc.sync.dma_start(out=w_ts["q"][:], in_=wq[:])
                    nc.sync.dma_start(out=w_ts["v"][:], in_=wv[:])
                    nc.sync.dma_start(out=xv_ts[0][:], in_=xv[0])
                else:
                    for d2 in range(1, DT):
                        nc.sync.dma_start(out=xv_ts[d2][:], in_=xv[d2])
                    nc.sync.dma_start(out=wo_sb[:], in_=wo[:])
                for fi in range(FT):
                    for tb in range(4):
                        if tb % 2 == 0:
                            nc.vector.tensor_scalar_add(
                                out=dst[:, fi, tb * 512:(tb + 1) * 512],
                                in0=banks[fi * 4 + tb][:],
                                scalar1=b_sb[name][:, fi:fi + 1],
                            )
                        else:
                            nc.scalar.activation(
                                out=dst[:, fi, tb * 512:(tb + 1) * 512],
                                in_=banks[fi * 4 + tb][:],
                                func=AFT.Identity,
                                bias=b_sb[name][:, fi:fi + 1],
                            )

            warmup = const.tile([1, 8], F32, tag="wrm")
            nc.vector.memset(warmup[:], 0.0)
            nc.scalar.activation(warmup[:], warmup[:], AFT.Exp)
            with tc.tile_pool(name="psA", bufs=8, space="PSUM") as psA:
                proj("k", xk, k_sb, psA)

            with tc.tile_pool(name="psA2", bufs=8, space="PSUM") as psA2:
                proj("q", xq, q_sb, psA2)

            # V^T projection (x stationary, w moving); xv lands last
            with tc.tile_pool(name="psV", bufs=2, space="PSUM") as psVp:
                for kt in range(KT):
                    vps = psVp.tile([128, F], F32, tag="v", bufs=2, name="vps")
                    for dt in range(DT):
                        nc.tensor.matmul(
                            vps[:], xv_ts[dt][:, kt * 128:(kt + 1) * 128],
                            w_ts["v"][:, dt, :],
                            start=(dt == 0), stop=(dt == DT - 1),
                        )
                    nc.vector.tensor_tensor(
                        out=vt_sb[:, kt, :], in0=vps[:], in1=bv_bc[:],
                        op=ALU.add)

            if KDBG:
                nc.sync.dma_start(out=dbg_k[:], in_=k_sb[:])
                nc.sync.dma_start(out=dbg_q[:], in_=q_sb[:])

            # ---- phase B: quarter-major attention + interleaved out-proj ----
            psB_stack = tc.tile_pool(name="psS", bufs=2, space="PSUM")
            psS = psB_stack.__enter__()
            psC_cm = tc.tile_pool(name="psC", bufs=1, space="PSUM")
            psC = psC_cm.__enter__()
            psD_cm = tc.tile_pool(name="psD", bufs=1, space="PSUM")
            psD = psD_cm.__enter__()

            def outproj_piece(tt, j):
                ob = psO.tile([128, 512], F32, tag="ob", name="ob")
                js = slice(j * 512, (j + 1) * 512)
                for fi in range(FT):
                    nc.tensor.matmul(
                        ob[:], ctx_sb[:, fi, tt * 128:(tt + 1) * 128],
                        wo_sb[:, fi, js], start=(fi == 0), stop=(fi == FT - 1))
                o_t = opool.tile([128, 512], BF16, tag="o", name="o_t")
                nc.vector.tensor_copy(o_t[:], ob[:])
                nc.sync.dma_start(out=out[tt * 128:(tt + 1) * 128, js],
                                  in_=o_t[:])

            def attn_quarter(P, Q, pieces=()):
                h0, h1 = 2 * P, 2 * P + 1
                qs = slice(Q * 512, (Q + 1) * 512)
                d_ps = psD.tile([128, 512], F32, tag="d", name="d_ps")
                ctx_ps = psC.tile([128, 512], F32, tag="ctx", name="ctx_ps")
                pieces = list(pieces)

                def scores(kt):
                    kts = slice(kt * 128, (kt + 1) * 128)
                    s2 = psS.tile([128, 1024], F32, tag="s", name="s2")
                    nc.tensor.matmul(s2[:, 0:512], k_sb[0:64, P, kts],
                                     q_sb[0:64, P, qs], start=True, stop=True)
                    nc.tensor.matmul(s2[:, 512:1024], k_sb[64:128, P, kts],
                                     q_sb[64:128, P, qs], start=True, stop=True)
                    return s2

                def consume(kt, p2):
                    # d + ctx for kt (exp(kt) finished >= one period ago)
                    st, sp = (kt == 0), (kt == KT - 1)
                    nc.tensor.matmul(d_ps[0:64, :], ones_bf[:],
                                     p2[:, 0:512], start=st, stop=sp)
                    nc.tensor.matmul(d_ps[64:128, :], ones_bf[:],
                                     p2[:, 512:1024], start=st, stop=sp,
                                     skip_group_check=True)
                    nc.tensor.matmul(ctx_ps[0:64, :],
                                     vt_sb[:, kt, 64 * h0:64 * h0 + 64],
                                     p2[:, 0:512], start=st, stop=sp)
                    nc.tensor.matmul(ctx_ps[64:128, :],
                                     vt_sb[:, kt, 64 * h1:64 * h1 + 64],
                                     p2[:, 512:1024], start=st, stop=sp,
                                     skip_group_check=True)

                s2 = scores(0)
                prev = None  # (kt, p2) whose d/ctx are deferred one iter
                for kt in range(KT):
                    p2 = ppool.tile([128, 1024], BF16, tag="p", name="p2")
                    if kt in DVE_KTS:
                        nc.vector.tensor_scalar(
                            out=p2.bitcast(I16)[:], in0=s2[:], scalar1=EXP_A,
                            scalar2=EXP_B, op0=ALU.mult, op1=ALU.add)
                    else:
                        nc.scalar.activation(p2[:], s2[:], AFT.Exp)
                    if kt + 1 < KT:
                        s2 = scores(kt + 1)
                    if pieces and kt % 4 == 3:
                        outproj_piece(*pieces.pop(0))
                    if prev is not None:
                        consume(*prev)
                    prev = (kt, p2)
                consume(*prev)
                for piece in pieces:
                    outproj_piece(*piece)
                # normalize: M=64 denominators already span each head's 64
                # partitions -> reciprocal + aligned multiplies, no broadcast
                linv = norm.tile([128, 512], F32, tag="linv", name="linv")
                nc.vector.reciprocal_approx_fast(linv[:], d_ps[:])
                nc.vector.tensor_tensor(
                    out=ctx_sb[0:64, P, qs], in0=ctx_ps[0:64, :],
                    in1=linv[0:64, :], op=ALU.mult)
                nc.vector.tensor_tensor(
                    out=ctx_sb[64:128, P, qs], in0=ctx_ps[64:128, :],
                    in1=linv[64:128, :], op=ALU.mult)

            psO_cm = tc.tile_pool(name="psO", bufs=2, space="PSUM")
            psO = psO_cm.__enter__()
            for Q in range(NQ):
                if Q == 0:
                    prev = []
                else:
                    prev = [(tt, j) for tt in range(4 * (Q - 1), 4 * Q)
                            for j in range(2)]
                attn_quarter(0, Q, prev[:4])
                attn_quarter(1, Q, prev[4:])
            # last quarter's out-projection tail (psS banks are free now);
            # copies alternate ACT/DVE to halve the drain latency
            for tt in range(4 * (NQ - 1), 4 * NQ):
                for j in range(2):
                    obw = psS.tile([128, 1024], F32, tag="s", name="s2")
                    ob = obw[:, 0:512]
                    js = slice(j * 512, (j + 1) * 512)
                    for fi in range(FT):
                        nc.tensor.matmul(
                            ob, ctx_sb[:, fi, tt * 128:(tt + 1) * 128],
                            wo_sb[:, fi, js], start=(fi == 0),
                            stop=(fi == FT - 1))
                    o_t = opool.tile([128, 512], BF16, tag="o", name="o_t")
                    if j == 0:
                        nc.scalar.copy(out=o_t[:], in_=ob)
                    else:
                        nc.vector.tensor_copy(o_t[:], ob)
                    nc.sync.dma_start(out=out[tt * 128:(tt + 1) * 128, js],
                                      in_=o_t[:])

            if KDBG:
                nc.sync.dma_start(out=dbg_vt[:], in_=vt_sb[:])
                nc.sync.dma_start(out=dbg_ctx[:], in_=ctx_sb[:])
            psO_cm.__exit__(None, None, None)
            psD_cm.__exit__(None, None, None)
            psC_cm.__exit__(None, None, None)
            psB_stack.__exit__(None, None, None)

    nc.compile()
    return nc


def get_program():
    if "nc" not in _CACHE:
        _CACHE["nc"] = _build()
    return _CACHE["nc"]


def _bf(a):
    import ml_dtypes
    return a.astype(ml_dtypes.bfloat16)


def prep_in_maps(query_tensor, key_tensor, value_tensor, w_q, b_q, w_k, b_k,
                 w_v, b_v, w_out, b_out):
    """Per-core input dicts. Core c: batch c//4, feature rows [256*(c%4), ...)."""
    f32 = np.float32
    scale = f32(1.0 / np.sqrt(DK))

    def xt(x, b):  # [S, D] -> [DT, 128, S]
        return _bf(np.ascontiguousarray(
            np.asarray(x[b], f32).T.reshape(DT, 128, S)))

    xs = {"xq_t": [xt(query_tensor, b) for b in range(B)],
          "xk_t": [xt(key_tensor, b) for b in range(B)],
          "xv_t": [xt(value_tensor, b) for b in range(B)]}

    def wt(w, g, s=f32(1.0)):  # rows [256g, 256g+256) of w -> [128, DT, F]
        sl = np.asarray(w[256 * g:256 * (g + 1), :], f32) * s  # [F, D]
        return _bf(np.ascontiguousarray(
            sl.T.reshape(DT, 128, F).transpose(1, 0, 2)))

    def bt(b_, g, s=f32(1.0)):  # [128, FT]
        sl = np.asarray(b_[256 * g:256 * (g + 1)], f32) * s
        return np.ascontiguousarray(sl.reshape(FT, 128).T)

    def wot(w, g):  # cols [256g, 256g+256) of w_out -> [128, FT, D]
        sl = np.asarray(w[:, 256 * g:256 * (g + 1)], f32)  # [D, F]
        return _bf(np.ascontiguousarray(
            sl.T.reshape(FT, 128, D).transpose(1, 0, 2)))

    in_maps = []
    for c in range(N_CORES):
        b, g = divmod(c, GROUPS)
        bv_sl = np.asarray(b_v[256 * g:256 * (g + 1)], f32).reshape(1, F)
        in_maps.append({
            "xq_t": xs["xq_t"][b], "xk_t": xs["xk_t"][b], "xv_t": xs["xv_t"][b],
            "wq_t": wt(w_q, g, scale), "wk_t": wt(w_k, g), "wv_t": wt(w_v, g),
            "bq": bt(b_q, g, scale), "bk": bt(b_k, g),
            "bv_row": np.ascontiguousarray(bv_sl),
            "wo_t": wot(w_out, g),
        })
    return in_maps


def kernel(query_tensor, key_tensor, value_tensor, w_q, b_q, w_k, b_k,
           w_v, b_v, w_out, b_out):
    global LAST_RESULTS
    nc = get_program()
    in_maps = prep_in_maps(query_tensor, key_tensor, value_tensor, w_q, b_q,
                           w_k, b_k, w_v, b_v, w_out, b_out)
    res = run_bass_kernel_spmd(nc, in_maps, list(range(N_CORES)),
                               tmpdir=os.environ.get("BASS_TMPDIR"))
    LAST_RESULTS = res
    b_out = np.asarray(b_out, np.float32)
    out = np.empty((B, S, D), np.float32)
    for b in range(B):
        import ml_dtypes
        acc = np.asarray(res.results[4 * b]["out_p"]).view(ml_dtypes.bfloat16).astype(np.float32)
        for g in range(1, GROUPS):
            acc = acc + np.asarray(res.results[4 * b + g]["out_p"]).view(
                ml_dtypes.bfloat16).astype(np.float32)
        out[b] = acc + b_out
    return out
